# revision 4
# baseline (speedup 1.0000x reference)
import sys

sys.path.insert(0, "/opt/trn_rl_repo")

import numpy as np
import concourse.bass as bass
import concourse.bacc as bacc
import concourse.mybir as mybir
import concourse.tile as tile
from concourse.bass_utils import run_bass_kernel_spmd
from concourse.library_config import mlp

P = 128
N = 20000
E = 320000
G = 64
DX = 128
DH = 128
DE = 64
NCORES = 8
NS = N // NCORES            # nodes per core (2500)
NT = (NS + P - 1) // P      # node tiles per core (20)
NSP = NT * P                # padded nodes per core (2560)
NPAD = NCORES * NSP         # padded table rows (20480)
ASLAB = 2048                # stage-A slab width
ELEM = 192                  # gather row width in f32 (768B, %256)

f32 = mybir.dt.float32
i16 = mybir.dt.int16


def _host_prep(x, hidden, edge_attr, W_emb, b_emb, w_agg, W_upd, b_upd,
               w_ro, W_score, b_score, edge_index, batch):
    rol = np.asarray(edge_index[0], dtype=np.int64)
    col = np.asarray(edge_index[1], dtype=np.int64)
    x = np.asarray(x, dtype=np.float32)
    hidden = np.asarray(hidden, dtype=np.float32)
    edge_attr = np.asarray(edge_attr, dtype=np.float32)
    batch = np.asarray(batch, dtype=np.int64)

    perm = np.argsort(col, kind="stable")
    rol_s = rol[perm]
    col_s = col[perm]
    ea_s = edge_attr[perm]

    # edges sorted by col -> contiguous slice per (core, node-tile)
    bounds = np.searchsorted(col_s, np.arange(0, NCORES * NS + 1))
    grp_lo = np.empty((NCORES, NT), dtype=np.int64)
    grp_hi = np.empty((NCORES, NT), dtype=np.int64)
    for k in range(NCORES):
        for j in range(NT):
            lo_node = k * NS + j * P
            hi_node = min(k * NS + (j + 1) * P, (k + 1) * NS)
            grp_lo[k, j] = bounds[lo_node]
            grp_hi[k, j] = bounds[hi_node]
    cnt = grp_hi - grp_lo
    T_max = max(int(np.max((cnt + P - 1) // P)), 1)
    EPT = T_max * P           # padded edges per node tile
    EPC = NT * EPT            # padded edges per core

    # rol index i of a node tile sits at gather position i -> (i%128, i//128);
    # idx tensor layout: idx i at [i % 16, i // 16], tiled to 128 partitions
    attT = np.zeros((NCORES, DE, EPC), dtype=np.float32)
    idx16 = np.zeros((NCORES, NT, P, EPT // 16), dtype=np.int16)
    colloc = np.full((NCORES, NT, P, T_max), -1.0, dtype=np.float32)
    for k in range(NCORES):
        for j in range(NT):
            lo, hi = grp_lo[k, j], grp_hi[k, j]
            n = int(hi - lo)
            base = j * EPT
            if n:
                attT[k, :, base:base + n] = ea_s[lo:hi].T
            r = np.zeros(EPT, dtype=np.int64)
            r[:n] = rol_s[lo:hi]
            # map global node id -> padded table row
            rr = (r // NS) * NSP + (r % NS)
            blk = rr.reshape(EPT // 16, 16).T.astype(np.int16)
            idx16[k, j] = np.tile(blk, (8, 1))
            cl = np.full(EPT, -1.0, dtype=np.float32)
            cl[:n] = (col_s[lo:hi] - (k * NS + j * P)).astype(np.float32)
            colloc[k, j] = cl.reshape(T_max, P).T                  # [P, T_max]

    xT_full = np.zeros((P, NPAD), dtype=np.float32)
    hT_full = np.zeros((P, NPAD), dtype=np.float32)
    for k in range(NCORES):
        xT_full[:, k * NSP:k * NSP + NS] = x[k * NS:(k + 1) * NS].T
        hT_full[:, k * NSP:k * NSP + NS] = hidden[k * NS:(k + 1) * NS].T

    bmat = np.zeros((NCORES, NT, P, G), dtype=np.float32)
    for k in range(NCORES):
        b = batch[k * NS:(k + 1) * NS]
        oh = np.zeros((NSP, G), dtype=np.float32)
        oh[np.arange(NS), b] = 1.0
        bmat[k] = oh.reshape(NT, P, G)

    W_emb = np.asarray(W_emb, dtype=np.float32)
    w_agg = np.asarray(w_agg, dtype=np.float32)
    W_upd = np.asarray(W_upd, dtype=np.float32)
    w_ro = np.asarray(w_ro, dtype=np.float32)

    weights = dict(
        WxA=np.ascontiguousarray(
            np.concatenate([W_emb[0:128], w_agg[0:128]], axis=1)),      # [128,129]
        WhA=np.ascontiguousarray(
            np.concatenate([W_emb[128:256], w_agg[128:256]], axis=1)),  # [128,129]
        bA=np.concatenate([np.asarray(b_emb, np.float32),
                           np.zeros(1, np.float32)])[None, :],          # [1,129]
        WqA=np.ascontiguousarray(
            np.concatenate([W_emb[256:320], w_agg[512:576]], axis=1)),  # [64,129]
        Wuh=np.ascontiguousarray(W_upd[0:128]),
        Wuc=np.ascontiguousarray(W_upd[128:256]),
        Wux=np.ascontiguousarray(W_upd[256:384]),
        bu=np.asarray(b_upd, np.float32)[None, :],                      # [1,128]
        wron=np.ascontiguousarray(w_ro[0:128]),
        wrox=np.ascontiguousarray(w_ro[128:256]),                       # [128,1]
        Wsc=np.asarray(W_score, np.float32),                            # [128,1]
        bsc=np.full((1, G), float(np.asarray(b_score).reshape(-1)[0]),
                    np.float32),                                        # [1,G]
        iota=np.tile(np.arange(P, dtype=np.float32), (P, 1)),           # [128,128]
        ident=np.eye(P, dtype=np.float32),                              # [128,128]
    )

    in_maps = []
    for k in range(NCORES):
        m = dict(weights)
        m["xT_full"] = xT_full
        m["hT_full"] = hT_full
        m["xT_sl"] = np.ascontiguousarray(xT_full[:, k * NSP:(k + 1) * NSP])
        m["hT_sl"] = np.ascontiguousarray(hT_full[:, k * NSP:(k + 1) * NSP])
        m["h_sl"] = np.ascontiguousarray(
            np.vstack([hidden[k * NS:(k + 1) * NS],
                       np.zeros((NSP - NS, DH), np.float32)]))
        m["attT"] = attT[k]
        m["idx16"] = idx16[k]
        m["colloc"] = colloc[k]
        m["bmat"] = bmat[k]
        in_maps.append(m)
    return in_maps, T_max


def _build_nc(T_max):
    EPT = T_max * P
    EPC = NT * EPT
    NB = (T_max + 3) // 4       # q PSUM banks (4 tiles of 128 per bank)
    nc = bacc.Bacc("TRN2", target_bir_lowering=False, debug=False,
                   num_devices=NCORES)

    ei = lambda nm, sh, dt=f32: nc.dram_tensor(nm, sh, dt, kind="ExternalInput")
    xT_full = ei("xT_full", [P, NPAD])
    hT_full = ei("hT_full", [P, NPAD])
    xT_sl = ei("xT_sl", [P, NSP])
    hT_sl = ei("hT_sl", [P, NSP])
    h_sl = ei("h_sl", [NSP, DH])
    attT = ei("attT", [DE, EPC])
    idx16 = ei("idx16", [NT, P, EPT // 16], i16)
    colloc = ei("colloc", [NT, P, T_max])
    bmat = ei("bmat", [NT, P, G])
    WxA = ei("WxA", [128, 129])
    WhA = ei("WhA", [128, 129])
    bA = ei("bA", [1, 129])
    WqA = ei("WqA", [64, 129])
    Wuh = ei("Wuh", [128, 128])
    Wuc = ei("Wuc", [128, 128])
    Wux = ei("Wux", [128, 128])
    bu = ei("bu", [1, 128])
    wron = ei("wron", [128, 1])
    wrox = ei("wrox", [128, 1])
    Wsc = ei("Wsc", [128, 1])
    bsc = ei("bsc", [1, G])
    iota = ei("iota", [P, P])
    ident = ei("ident", [P, P])

    nf_out = nc.dram_tensor("nf_out", [NSP, DH], f32, kind="ExternalOutput")
    conf_out = nc.dram_tensor("conf_out", [1, G], f32, kind="ExternalOutput")

    AF = mybir.ActivationFunctionType
    OP = mybir.AluOpType

    with tile.TileContext(nc) as tc:
        with (
            tc.tile_pool(name="const", bufs=1) as cpool,
            tc.tile_pool(name="aslab", bufs=2) as apool,
            tc.tile_pool(name="asb", bufs=4) as asb,
            tc.tile_pool(name="edge", bufs=2) as epool,
            tc.tile_pool(name="gat", bufs=2) as gpool,
            tc.tile_pool(name="node", bufs=3) as npool,
            tc.tile_pool(name="psM", bufs=2, space="PSUM") as psM,
            tc.tile_pool(name="psQ", bufs=1, space="PSUM") as psQ,
            tc.tile_pool(name="psC", bufs=1, space="PSUM") as psC,
            tc.tile_pool(name="dram", bufs=1, space="DRAM") as dpool,
        ):
            nc.gpsimd.load_library(mlp)

            def cload(src, sh, dt=f32):
                t = cpool.tile(sh, dt, tag=src.name)
                nc.sync.dma_start(t[:], src[:])
                return t

            WxA_t = cload(WxA, [128, 129])
            WhA_t = cload(WhA, [128, 129])
            bA_t = cload(bA, [1, 129])
            WqA_t = cload(WqA, [64, 129])
            Wuh_t = cload(Wuh, [128, 128])
            Wuc_t = cload(Wuc, [128, 128])
            Wux_t = cload(Wux, [128, 128])
            bu_t = cload(bu, [1, 128])
            wron_t = cload(wron, [128, 1])
            wrox_t = cload(wrox, [128, 1])
            Wsc_t = cload(Wsc, [128, 1])
            bsc_t = cload(bsc, [1, G])
            iota_t = cload(iota, [P, P])
            ident_t = cload(ident, [P, P])
            ones_t = cpool.tile([1, 128], f32, tag="ones")
            nc.vector.memset(ones_t[:], 1.0)
            gf_a = cpool.tile([G, 129], f32, tag="gfa")
            gf_b = cpool.tile([G, 129], f32, tag="gfb")

            table = dpool.tile([NPAD, ELEM], f32)

            # ---------------- stage A: node table [P | a_src] ----------------
            for s in range(NPAD // ASLAB):
                xs = apool.tile([P, ASLAB], f32, tag="xs")
                hs = apool.tile([P, ASLAB], f32, tag="hs")
                nc.sync.dma_start(xs[:], xT_full[:, s * ASLAB:(s + 1) * ASLAB])
                nc.sync.dma_start(hs[:], hT_full[:, s * ASLAB:(s + 1) * ASLAB])
                for t in range(ASLAB // P):
                    ps = psM.tile([P, 129], f32, space="PSUM", tag="misc")
                    nc.tensor.matmul(ps[:], xs[:, t * P:(t + 1) * P], WxA_t[:],
                                     start=True, stop=False)
                    nc.tensor.matmul(ps[:], hs[:, t * P:(t + 1) * P], WhA_t[:],
                                     start=False, stop=False)
                    nc.tensor.matmul(ps[:], ones_t[:1, :], bA_t[:],
                                     start=False, stop=True)
                    sa = asb.tile([P, 129], f32, tag="sa")
                    nc.scalar.activation(sa[:], ps[:], AF.Copy)
                    r0 = s * ASLAB + t * P
                    nc.sync.dma_start(table[r0:r0 + P, 0:129], sa[:])

            # ---------------- stage B: edge pipeline ----------------
            for j in range(NT):
                idx_t = epool.tile([P, EPT // 16], i16, tag="idx")
                nc.sync.dma_start(idx_t[:], idx16[j])
                cj_t = epool.tile([P, T_max], f32, tag="cj")
                nc.sync.dma_start(cj_t[:], colloc[j])
                at_t = epool.tile([DE, EPT], f32, tag="at")
                nc.sync.dma_start(at_t[:], attT[:, j * EPT:(j + 1) * EPT])
                g_t = gpool.tile([P, T_max, ELEM], f32, tag="g")
                nc.gpsimd.dma_gather(g_t[:], table[:, :], idx_t[:], EPT, EPT, ELEM,
                                     single_packet=False)

                # q matmuls: 4 tiles of [128,128] per PSUM bank; ae into c_ps
                q_ps = psQ.tile([P, NB * 512], f32, space="PSUM", tag="q")
                c_ps = psC.tile([P, 512], f32, space="PSUM", tag="c")
                for t in range(T_max):
                    off = (t // 4) * 512 + (t % 4) * 128
                    nc.tensor.matmul(q_ps[:, off:off + 128],
                                     at_t[:, t * P:(t + 1) * P], WqA_t[:, 0:128],
                                     start=True, stop=True)
                    nc.tensor.matmul(c_ps[:, 384 + t:385 + t],
                                     at_t[:, t * P:(t + 1) * P], WqA_t[:, 128:129],
                                     start=True, stop=True, skip_group_check=True)

                # V = P[rol] + q   (wide adds over 4-tile bank groups)
                v_t = gpool.tile([P, T_max * P], f32, tag="v")
                n4 = T_max // 4
                r4 = T_max - n4 * 4
                if n4:
                    nc.vector.tensor_tensor(
                        out=v_t[:, 0:n4 * 512].rearrange(
                            "p (a b c) -> p a b c", a=n4, b=4),
                        in0=q_ps[:, 0:n4 * 512].rearrange(
                            "p (a b c) -> p a b c", a=n4, b=4),
                        in1=g_t[:, 0:n4 * 4, 0:128].rearrange(
                            "p (a b) c -> p a b c", a=n4),
                        op=OP.add)
                if r4:
                    nc.vector.tensor_tensor(
                        out=v_t[:, n4 * 512:].rearrange("p (a c) -> p a c", a=r4),
                        in0=q_ps[:, n4 * 512:n4 * 512 + r4 * 128].rearrange(
                            "p (a c) -> p a c", a=r4),
                        in1=g_t[:, n4 * 4:, 0:128],
                        op=OP.add)

                # att = a_src + ae ; alpha = exp(att)
                att_t = epool.tile([P, T_max], f32, tag="att")
                nc.vector.tensor_tensor(
                    out=att_t[:], in0=g_t[:, :, 128],
                    in1=c_ps[:, 384:384 + T_max], op=OP.add)
                al_t = epool.tile([P, T_max], f32, tag="al")
                nc.scalar.activation(al_t[:], att_t[:], AF.Exp)

                # S = onehot(col_local), one wide op
                s_t = gpool.tile([P, T_max * P], f32, tag="s")
                nc.vector.tensor_tensor(
                    out=s_t[:].rearrange("p (a b) -> p a b", a=T_max),
                    in0=iota_t[:].unsqueeze(1).broadcast_to([P, T_max, P]),
                    in1=cj_t[:].to_broadcast([P, T_max, P]),
                    op=OP.is_equal)

                # Vt = relu(alpha * V); scatter-matmuls into [C | z]
                vt_t = gpool.tile([P, T_max * P], f32, tag="vt")
                for t in range(T_max):
                    nc.scalar.activation(vt_t[:, t * P:(t + 1) * P],
                                         v_t[:, t * P:(t + 1) * P], AF.Relu,
                                         scale=al_t[:, t:t + 1])
                    # start=True clears the whole bank's has_written bits, so
                    # only the first matmul touching this bank may set it.
                    nc.tensor.matmul(c_ps[:, 0:128], s_t[:, t * P:(t + 1) * P],
                                     vt_t[:, t * P:(t + 1) * P],
                                     start=(t == 0), stop=False,
                                     skip_group_check=True)
                    nc.tensor.matmul(c_ps[:, 128:129], s_t[:, t * P:(t + 1) * P],
                                     al_t[:, t:t + 1],
                                     start=False, stop=(t == T_max - 1),
                                     skip_group_check=True)

                # ---------------- node stage ----------------
                zr = npool.tile([P, 1], f32, tag="zr")
                nc.vector.tensor_scalar_add(zr[:], c_ps[:, 128:129], 1e-16)
                nc.vector.reciprocal(zr[:], zr[:])
                cn = npool.tile([P, 128], f32, tag="cn")
                nc.vector.tensor_scalar_mul(cn[:], c_ps[:, 0:128], zr[:])

                ct_ps = psM.tile([P, 128], f32, space="PSUM", tag="misc")
                nc.tensor.transpose(ct_ps[:], cn[:], ident_t[:])
                ct = npool.tile([P, 128], f32, tag="ct")
                nc.scalar.activation(ct[:], ct_ps[:], AF.Copy)

                ht_j = npool.tile([P, 128], f32, tag="htj")
                nc.sync.dma_start(ht_j[:], hT_sl[:, j * P:(j + 1) * P])
                xt_j = npool.tile([P, 128], f32, tag="xtj")
                nc.sync.dma_start(xt_j[:], xT_sl[:, j * P:(j + 1) * P])
                h_j = npool.tile([P, 128], f32, tag="hj")
                nc.sync.dma_start(h_j[:], h_sl[j * P:(j + 1) * P, :])

                g_ps = psM.tile([P, 128], f32, space="PSUM", tag="misc")
                nc.tensor.matmul(g_ps[:], ht_j[:], Wuh_t[:], start=True, stop=False)
                nc.tensor.matmul(g_ps[:], ct[:], Wuc_t[:], start=False, stop=False)
                nc.tensor.matmul(g_ps[:], xt_j[:], Wux_t[:], start=False, stop=False)
                nc.tensor.matmul(g_ps[:], ones_t[:1, :], bu_t[:],
                                 start=False, stop=True)
                gate = npool.tile([P, 128], f32, tag="gate")
                nc.scalar.activation(gate[:], g_ps[:], AF.Sigmoid)

                d_t = npool.tile([P, 128], f32, tag="d")
                nc.vector.tensor_tensor(out=d_t[:], in0=cn[:], in1=h_j[:],
                                        op=OP.subtract)
                nf1 = npool.tile([P, 128], f32, tag="nf1")
                nc.vector.tensor_tensor(out=nf1[:], in0=gate[:], in1=d_t[:],
                                        op=OP.mult)
                nf2 = npool.tile([P, 128], f32, tag="nf2")
                nc.vector.tensor_tensor(out=nf2[:], in0=nf1[:], in1=h_j[:],
                                        op=OP.add)
                nc.sync.dma_start(nf_out[j * P:(j + 1) * P, :], nf2[:])

                nft_ps = psM.tile([P, 128], f32, space="PSUM", tag="misc")
                nc.tensor.transpose(nft_ps[:], nf2[:], ident_t[:])
                nft = npool.tile([P, 128], f32, tag="nft")
                nc.scalar.activation(nft[:], nft_ps[:], AF.Copy)

                ro_ps = psM.tile([P, 1], f32, space="PSUM", tag="misc")
                nc.tensor.matmul(ro_ps[:], nft[:], wron_t[:], start=True, stop=False)
                nc.tensor.matmul(ro_ps[:], xt_j[:], wrox_t[:], start=False, stop=True)
                ero = npool.tile([P, 1], f32, tag="ero")
                nc.scalar.activation(ero[:], ro_ps[:], AF.Exp)

                r_t = npool.tile([P, 129], f32, tag="rt")
                nc.vector.tensor_scalar_mul(r_t[:, 0:128], nf2[:], ero[:])
                nc.vector.tensor_copy(r_t[:, 128:129], ero[:])

                bj = npool.tile([P, G], f32, tag="bj")
                nc.sync.dma_start(bj[:], bmat[j])
                gfj_ps = psM.tile([G, 129], f32, space="PSUM", tag="misc")
                nc.tensor.matmul(gfj_ps[:], bj[:], r_t[:], start=True, stop=True)
                if j == 0:
                    nc.vector.tensor_copy(gf_b[:], gfj_ps[:])
                elif j % 2 == 1:
                    nc.vector.tensor_tensor(out=gf_a[:], in0=gf_b[:],
                                            in1=gfj_ps[:], op=OP.add)
                else:
                    nc.vector.tensor_tensor(out=gf_b[:], in0=gf_a[:],
                                            in1=gfj_ps[:], op=OP.add)

            gf_fin = gf_a if NT % 2 == 0 else gf_b

            # ---------------- readout tail ----------------
            ar_in = dpool.tile([G, 129], f32)
            ar_out = dpool.tile([G, 129], f32)
            nc.sync.dma_start(ar_in[:], gf_fin[:])
            nc.gpsimd.collective_compute(
                "AllReduce", OP.add, replica_groups=[list(range(NCORES))],
                ins=[ar_in.opt()], outs=[ar_out.opt()])
            gfr = npool.tile([G, 129], f32, tag="gfr")
            nc.sync.dma_start(gfr[:], ar_out[:])

            zg = npool.tile([G, 1], f32, tag="zg")
            nc.vector.tensor_scalar_add(zg[:], gfr[:, 128:129], 1e-16)
            nc.vector.reciprocal(zg[:], zg[:])
            gf = npool.tile([G, 128], f32, tag="gf")
            nc.vector.tensor_scalar_mul(gf[:], gfr[:, 0:128], zg[:])

            gft_ps = psM.tile([P, G], f32, space="PSUM", tag="misc")
            nc.tensor.transpose(gft_ps[:], gf[:], ident_t[0:G, 0:G])
            gft = npool.tile([P, G], f32, tag="gft")
            nc.scalar.activation(gft[:], gft_ps[:], AF.Copy)

            cf_ps = psM.tile([1, G], f32, space="PSUM", tag="misc")
            nc.tensor.matmul(cf_ps[:], Wsc_t[:], gft[:], start=True, stop=False)
            nc.tensor.matmul(cf_ps[:], ones_t[:1, :1], bsc_t[:],
                             start=False, stop=True)
            conf = npool.tile([1, G], f32, tag="conf")
            nc.scalar.activation(conf[:], cf_ps[:], AF.Sigmoid)
            nc.sync.dma_start(conf_out[:], conf[:])

    nc.compile()
    return nc


_CACHE = {}


def kernel(**inputs):
    num_graphs = int(np.asarray(inputs["num_graphs"]))
    assert num_graphs == G
    in_maps, T_max = _host_prep(
        inputs["x"], inputs["hidden_node_feat"], inputs["edge_attr"],
        inputs["W_emb"], inputs["b_emb"], inputs["w_agg"], inputs["W_upd"],
        inputs["b_upd"], inputs["w_ro"], inputs["W_score"], inputs["b_score"],
        inputs["edge_index"], inputs["batch"])
    if T_max not in _CACHE:
        _CACHE[T_max] = _build_nc(T_max)
    nc = _CACHE[T_max]
    res = run_bass_kernel_spmd(nc, in_maps, core_ids=list(range(NCORES)))
    node_feat = np.concatenate(
        [res.results[k]["nf_out"][:NS] for k in range(NCORES)], axis=0)
    confidence = res.results[0]["conf_out"].reshape(G, 1)
    return node_feat, confidence


# revision 6
# speedup vs baseline: 11.4419x; 11.4419x over previous
import sys

sys.path.insert(0, "/opt/trn_rl_repo")

import contextlib
import numpy as np
import concourse.bass as bass
import concourse.bacc as bacc
import concourse.mybir as mybir
import concourse.tile as tile
from concourse.bass_utils import run_bass_kernel_spmd
from concourse.library_config import mlp

P = 128
N = 20000
E = 320000
G = 64
DX = 128
DH = 128
DE = 64
NCORES = 8
NS = N // NCORES            # nodes per core (2500)
NT = (NS + P - 1) // P      # node tiles per core (20)
NSP = NT * P                # padded nodes per core (2560)
NPAD = NCORES * NSP         # padded table rows (20480)
ASLAB = 2048                # stage-A slab width
ELEM = 192                  # gather row width in f32 (768B, %256)

f32 = mybir.dt.float32
i16 = mybir.dt.int16


def _host_prep(x, hidden, edge_attr, W_emb, b_emb, w_agg, W_upd, b_upd,
               w_ro, W_score, b_score, edge_index, batch):
    rol = np.asarray(edge_index[0], dtype=np.int64)
    col = np.asarray(edge_index[1], dtype=np.int64)
    x = np.asarray(x, dtype=np.float32)
    hidden = np.asarray(hidden, dtype=np.float32)
    edge_attr = np.asarray(edge_attr, dtype=np.float32)
    batch = np.asarray(batch, dtype=np.int64)

    perm = np.argsort(col, kind="stable")
    rol_s = rol[perm]
    col_s = col[perm]
    ea_s = edge_attr[perm]

    # edges sorted by col -> contiguous slice per (core, node-tile)
    bounds = np.searchsorted(col_s, np.arange(0, NCORES * NS + 1))
    grp_lo = np.empty((NCORES, NT), dtype=np.int64)
    grp_hi = np.empty((NCORES, NT), dtype=np.int64)
    for k in range(NCORES):
        for j in range(NT):
            lo_node = k * NS + j * P
            hi_node = min(k * NS + (j + 1) * P, (k + 1) * NS)
            grp_lo[k, j] = bounds[lo_node]
            grp_hi[k, j] = bounds[hi_node]
    cnt = grp_hi - grp_lo
    T_max = max(int(np.max((cnt + P - 1) // P)), 1)
    EPT = T_max * P           # padded edges per node tile
    EPC = NT * EPT            # padded edges per core

    # gather position i -> (partition i%128, block i//128);
    # idx tensor layout: idx i at [i % 16, i // 16], tiled to 128 partitions
    attT = np.zeros((NCORES, DE, EPC), dtype=np.float32)
    idx16 = np.zeros((NCORES, NT, P, EPT // 16), dtype=np.int16)
    colloc = np.full((NCORES, NT, P, T_max), -1.0, dtype=np.float32)
    for k in range(NCORES):
        for j in range(NT):
            lo, hi = grp_lo[k, j], grp_hi[k, j]
            n = int(hi - lo)
            base = j * EPT
            if n:
                attT[k, :, base:base + n] = ea_s[lo:hi].T
            r = np.zeros(EPT, dtype=np.int64)
            r[:n] = rol_s[lo:hi]
            # map global node id -> padded table row
            rr = (r // NS) * NSP + (r % NS)
            blk = rr.reshape(EPT // 16, 16).T.astype(np.int16)
            idx16[k, j] = np.tile(blk, (8, 1))
            cl = np.full(EPT, -1.0, dtype=np.float32)
            cl[:n] = (col_s[lo:hi] - (k * NS + j * P)).astype(np.float32)
            colloc[k, j] = cl.reshape(T_max, P).T                  # [P, T_max]

    xT_full = np.zeros((P, NPAD), dtype=np.float32)
    hT_full = np.zeros((P, NPAD), dtype=np.float32)
    for k in range(NCORES):
        xT_full[:, k * NSP:k * NSP + NS] = x[k * NS:(k + 1) * NS].T
        hT_full[:, k * NSP:k * NSP + NS] = hidden[k * NS:(k + 1) * NS].T

    bmat = np.zeros((NCORES, NT, P, G), dtype=np.float32)
    for k in range(NCORES):
        b = batch[k * NS:(k + 1) * NS]
        oh = np.zeros((NSP, G), dtype=np.float32)
        oh[np.arange(NS), b] = 1.0
        bmat[k] = oh.reshape(NT, P, G)

    W_emb = np.asarray(W_emb, dtype=np.float32)
    w_agg = np.asarray(w_agg, dtype=np.float32)
    W_upd = np.asarray(W_upd, dtype=np.float32)
    w_ro = np.asarray(w_ro, dtype=np.float32)

    weights = dict(
        WxA=np.ascontiguousarray(
            np.concatenate([W_emb[0:128], w_agg[0:128]], axis=1)),      # [128,129]
        WhA=np.ascontiguousarray(
            np.concatenate([W_emb[128:256], w_agg[128:256]], axis=1)),  # [128,129]
        bA=np.concatenate([np.asarray(b_emb, np.float32),
                           np.zeros(1, np.float32)])[None, :],          # [1,129]
        WqA=np.ascontiguousarray(
            np.concatenate([W_emb[256:320], w_agg[512:576]], axis=1)),  # [64,129]
        Wuh=np.ascontiguousarray(W_upd[0:128]),
        Wuc=np.ascontiguousarray(W_upd[128:256]),
        Wux=np.ascontiguousarray(W_upd[256:384]),
        bu=np.asarray(b_upd, np.float32)[None, :],                      # [1,128]
        wron=np.ascontiguousarray(w_ro[0:128]),
        wrox=np.ascontiguousarray(w_ro[128:256]),                       # [128,1]
        Wsc=np.asarray(W_score, np.float32),                            # [128,1]
        bsc=np.full((1, G), float(np.asarray(b_score).reshape(-1)[0]),
                    np.float32),                                        # [1,G]
        iota=np.tile(np.arange(P, dtype=np.float32), (P, 1)),           # [128,128]
        ident=np.eye(P, dtype=np.float32),                              # [128,128]
    )

    in_maps = []
    for k in range(NCORES):
        m = dict(weights)
        m["xT_full"] = xT_full
        m["hT_full"] = hT_full
        m["xT_sl"] = np.ascontiguousarray(xT_full[:, k * NSP:(k + 1) * NSP])
        m["hT_sl"] = np.ascontiguousarray(hT_full[:, k * NSP:(k + 1) * NSP])
        m["h_sl"] = np.ascontiguousarray(
            np.vstack([hidden[k * NS:(k + 1) * NS],
                       np.zeros((NSP - NS, DH), np.float32)]))
        m["attT"] = attT[k]
        m["idx16"] = idx16[k]
        m["colloc"] = colloc[k]
        m["bmat"] = bmat[k]
        in_maps.append(m)
    return in_maps, T_max


class _Env:
    pass


def _emit_body(e):
    """Stage A (node table) + stage B (edge pipeline + node stage)."""
    nc, T_max, EPT, NB = e.nc, e.T_max, e.EPT, e.NB
    AF = mybir.ActivationFunctionType
    OP = mybir.AluOpType

    # ---------------- stage A: node table [P | a_src] ----------------
    for s in range(NPAD // ASLAB):
        xs = e.apool.tile([P, ASLAB], f32, tag="xs")
        hs = e.apool.tile([P, ASLAB], f32, tag="hs")
        nc.sync.dma_start(xs[:], e.xT_full[:, s * ASLAB:(s + 1) * ASLAB])
        nc.sync.dma_start(hs[:], e.hT_full[:, s * ASLAB:(s + 1) * ASLAB])
        for t in range(ASLAB // P):
            ps = e.psM.tile([P, 129], f32, space="PSUM", tag="misc")
            nc.tensor.matmul(ps[:], xs[:, t * P:(t + 1) * P], e.WxA_t[:],
                             start=True, stop=False)
            nc.tensor.matmul(ps[:], hs[:, t * P:(t + 1) * P], e.WhA_t[:],
                             start=False, stop=False)
            nc.tensor.matmul(ps[:], e.ones_t[:1, :], e.bA_t[:],
                             start=False, stop=True)
            sa = e.asb.tile([P, 129], f32, tag="sa")
            nc.scalar.activation(sa[:], ps[:], AF.Copy)
            r0 = s * ASLAB + t * P
            nc.sync.dma_start(e.table[r0:r0 + P, 0:129], sa[:])

    # ---------------- stage B: edge pipeline ----------------
    for j in range(NT):
        idx_t = e.epool.tile([P, EPT // 16], i16, tag="idx")
        nc.sync.dma_start(idx_t[:], e.idx16[j])
        cj_t = e.epool.tile([P, T_max], f32, tag="cj")
        nc.sync.dma_start(cj_t[:], e.colloc[j])
        at_t = e.epool.tile([DE, EPT], f32, tag="at")
        nc.sync.dma_start(at_t[:], e.attT[:, j * EPT:(j + 1) * EPT])
        g_t = e.gpool.tile([P, T_max, ELEM], f32, tag="g")
        nc.gpsimd.dma_gather(g_t[:], e.table[:, :], idx_t[:], EPT, EPT, ELEM,
                             single_packet=False)

        # q matmuls: 4 tiles of [128,128] per PSUM bank; ae into c_ps spare
        q_ps = e.psQ.tile([P, NB * 512], f32, space="PSUM", tag="q")
        c_ps = e.psC.tile([P, 512], f32, space="PSUM", tag="c")
        for t in range(T_max):
            off = (t // 4) * 512 + (t % 4) * 128
            nc.tensor.matmul(q_ps[:, off:off + 128],
                             at_t[:, t * P:(t + 1) * P], e.WqA_t[:, 0:128],
                             start=True, stop=True)
            nc.tensor.matmul(c_ps[:, 384 + t:385 + t],
                             at_t[:, t * P:(t + 1) * P], e.WqA_t[:, 128:129],
                             start=True, stop=True, skip_group_check=True)

        # V = P[rol] + q   (wide adds over 4-tile bank groups)
        v_t = e.gpool.tile([P, T_max * P], f32, tag="v")
        n4 = T_max // 4
        r4 = T_max - n4 * 4
        if n4:
            nc.vector.tensor_tensor(
                out=v_t[:, 0:n4 * 512].rearrange(
                    "p (a b c) -> p a b c", a=n4, b=4),
                in0=q_ps[:, 0:n4 * 512].rearrange(
                    "p (a b c) -> p a b c", a=n4, b=4),
                in1=g_t[:, 0:n4 * 4, 0:128].rearrange(
                    "p (a b) c -> p a b c", a=n4),
                op=OP.add)
        if r4:
            nc.vector.tensor_tensor(
                out=v_t[:, n4 * 512:].rearrange("p (a c) -> p a c", a=r4),
                in0=q_ps[:, n4 * 512:n4 * 512 + r4 * 128].rearrange(
                    "p (a c) -> p a c", a=r4),
                in1=g_t[:, n4 * 4:, 0:128],
                op=OP.add)

        # att = a_src + ae ; alpha = exp(att)
        att_t = e.epool.tile([P, T_max], f32, tag="att")
        nc.vector.tensor_tensor(
            out=att_t[:], in0=g_t[:, :, 128],
            in1=c_ps[:, 384:384 + T_max], op=OP.add)
        al_t = e.epool.tile([P, T_max], f32, tag="al")
        nc.scalar.activation(al_t[:], att_t[:], AF.Exp)

        # S = onehot(col_local), one wide op
        s_t = e.gpool.tile([P, T_max * P], f32, tag="s")
        nc.vector.tensor_tensor(
            out=s_t[:].rearrange("p (a b) -> p a b", a=T_max),
            in0=e.iota_t[:].unsqueeze(1).broadcast_to([P, T_max, P]),
            in1=cj_t[:].to_broadcast([P, T_max, P]),
            op=OP.is_equal)

        # Vt = relu(alpha * V); scatter-matmuls into [C | z]
        vt_t = e.gpool.tile([P, T_max * P], f32, tag="vt")
        for t in range(T_max):
            nc.scalar.activation(vt_t[:, t * P:(t + 1) * P],
                                 v_t[:, t * P:(t + 1) * P], AF.Relu,
                                 scale=al_t[:, t:t + 1])
            # start=True clears the whole bank's has_written bits, so only
            # the first matmul touching this bank may set it.
            nc.tensor.matmul(c_ps[:, 0:128], s_t[:, t * P:(t + 1) * P],
                             vt_t[:, t * P:(t + 1) * P],
                             start=(t == 0), stop=False,
                             skip_group_check=True)
            nc.tensor.matmul(c_ps[:, 128:129], s_t[:, t * P:(t + 1) * P],
                             al_t[:, t:t + 1],
                             start=False, stop=(t == T_max - 1),
                             skip_group_check=True)

        # ---------------- node stage ----------------
        zr = e.npool.tile([P, 1], f32, tag="zr")
        nc.vector.tensor_scalar_add(zr[:], c_ps[:, 128:129], 1e-16)
        nc.vector.reciprocal(zr[:], zr[:])
        cn = e.npool.tile([P, 128], f32, tag="cn")
        nc.vector.tensor_scalar_mul(cn[:], c_ps[:, 0:128], zr[:])

        ct_ps = e.psM.tile([P, 128], f32, space="PSUM", tag="misc")
        nc.tensor.transpose(ct_ps[:], cn[:], e.ident_t[:])
        ct = e.npool.tile([P, 128], f32, tag="ct")
        nc.scalar.activation(ct[:], ct_ps[:], AF.Copy)

        ht_j = e.npool.tile([P, 128], f32, tag="htj")
        nc.sync.dma_start(ht_j[:], e.hT_sl[:, j * P:(j + 1) * P])
        xt_j = e.npool.tile([P, 128], f32, tag="xtj")
        nc.sync.dma_start(xt_j[:], e.xT_sl[:, j * P:(j + 1) * P])
        h_j = e.npool.tile([P, 128], f32, tag="hj")
        nc.sync.dma_start(h_j[:], e.h_sl[j * P:(j + 1) * P, :])

        g_ps = e.psM.tile([P, 128], f32, space="PSUM", tag="misc")
        nc.tensor.matmul(g_ps[:], ht_j[:], e.Wuh_t[:], start=True, stop=False)
        nc.tensor.matmul(g_ps[:], ct[:], e.Wuc_t[:], start=False, stop=False)
        nc.tensor.matmul(g_ps[:], xt_j[:], e.Wux_t[:], start=False, stop=False)
        nc.tensor.matmul(g_ps[:], e.ones_t[:1, :], e.bu_t[:],
                         start=False, stop=True)
        gate = e.npool.tile([P, 128], f32, tag="gate")
        nc.scalar.activation(gate[:], g_ps[:], AF.Sigmoid)

        d_t = e.npool.tile([P, 128], f32, tag="d")
        nc.vector.tensor_tensor(out=d_t[:], in0=cn[:], in1=h_j[:],
                                op=OP.subtract)
        nf1 = e.npool.tile([P, 128], f32, tag="nf1")
        nc.vector.tensor_tensor(out=nf1[:], in0=gate[:], in1=d_t[:],
                                op=OP.mult)
        nf2 = e.npool.tile([P, 128], f32, tag="nf2")
        nc.vector.tensor_tensor(out=nf2[:], in0=nf1[:], in1=h_j[:],
                                op=OP.add)
        nc.sync.dma_start(e.nf_out[j * P:(j + 1) * P, :], nf2[:])

        nft_ps = e.psM.tile([P, 128], f32, space="PSUM", tag="misc")
        nc.tensor.transpose(nft_ps[:], nf2[:], e.ident_t[:])
        nft = e.npool.tile([P, 128], f32, tag="nft")
        nc.scalar.activation(nft[:], nft_ps[:], AF.Copy)

        ro_ps = e.psM.tile([P, 1], f32, space="PSUM", tag="misc")
        nc.tensor.matmul(ro_ps[:], nft[:], e.wron_t[:], start=True, stop=False)
        nc.tensor.matmul(ro_ps[:], xt_j[:], e.wrox_t[:], start=False, stop=True)
        ero = e.npool.tile([P, 1], f32, tag="ero")
        nc.scalar.activation(ero[:], ro_ps[:], AF.Exp)

        r_t = e.npool.tile([P, 129], f32, tag="rt")
        nc.vector.tensor_scalar_mul(r_t[:, 0:128], nf2[:], ero[:])
        nc.vector.tensor_copy(r_t[:, 128:129], ero[:])

        bj = e.npool.tile([P, G], f32, tag="bj")
        nc.sync.dma_start(bj[:], e.bmat[j])
        gfj_ps = e.psM.tile([G, 129], f32, space="PSUM", tag="misc")
        nc.tensor.matmul(gfj_ps[:], bj[:], r_t[:], start=True, stop=True)
        if j == 0:
            nc.vector.tensor_copy(e.gf_b[:], gfj_ps[:])
        elif j % 2 == 1:
            nc.vector.tensor_tensor(out=e.gf_a[:], in0=e.gf_b[:],
                                    in1=gfj_ps[:], op=OP.add)
        else:
            nc.vector.tensor_tensor(out=e.gf_b[:], in0=e.gf_a[:],
                                    in1=gfj_ps[:], op=OP.add)


def _emit_tail(e):
    nc = e.nc
    AF = mybir.ActivationFunctionType
    OP = mybir.AluOpType
    gf_fin = e.gf_a if NT % 2 == 0 else e.gf_b

    ar_in = e.dpool.tile([G, 129], f32)
    ar_out = e.dpool.tile([G, 129], f32)
    nc.sync.dma_start(ar_in[:], gf_fin[:])
    nc.gpsimd.collective_compute(
        "AllReduce", OP.add, replica_groups=[list(range(NCORES))],
        ins=[ar_in.opt()], outs=[ar_out.opt()])
    gfr = e.npool.tile([G, 129], f32, tag="gfr")
    nc.sync.dma_start(gfr[:], ar_out[:])

    zg = e.npool.tile([G, 1], f32, tag="zg")
    nc.vector.tensor_scalar_add(zg[:], gfr[:, 128:129], 1e-16)
    nc.vector.reciprocal(zg[:], zg[:])
    gf = e.npool.tile([G, 128], f32, tag="gf")
    nc.vector.tensor_scalar_mul(gf[:], gfr[:, 0:128], zg[:])

    gft_ps = e.psM.tile([P, G], f32, space="PSUM", tag="misc")
    nc.tensor.transpose(gft_ps[:], gf[:], e.ident_t[0:G, 0:G])
    gft = e.npool.tile([P, G], f32, tag="gft")
    nc.scalar.activation(gft[:], gft_ps[:], AF.Copy)

    cf_ps = e.psM.tile([1, G], f32, space="PSUM", tag="misc")
    nc.tensor.matmul(cf_ps[:], e.Wsc_t[:], gft[:], start=True, stop=False)
    nc.tensor.matmul(cf_ps[:], e.ones_t[:1, :1], e.bsc_t[:],
                     start=False, stop=True)
    conf = e.npool.tile([1, G], f32, tag="conf")
    nc.scalar.activation(conf[:], cf_ps[:], AF.Sigmoid)
    nc.sync.dma_start(e.conf_out[:], conf[:])


def _build_nc(T_max, repeat=1):
    EPT = T_max * P
    NB = (T_max + 3) // 4       # q PSUM banks (4 tiles of 128 per bank)
    EPC = NT * EPT
    nc = bacc.Bacc("TRN2", target_bir_lowering=False, debug=False,
                   num_devices=NCORES)
    e = _Env()
    e.nc, e.T_max, e.EPT, e.NB = nc, T_max, EPT, NB

    ei = lambda nm, sh, dt=f32: nc.dram_tensor(nm, sh, dt, kind="ExternalInput")
    e.xT_full = ei("xT_full", [P, NPAD])
    e.hT_full = ei("hT_full", [P, NPAD])
    e.xT_sl = ei("xT_sl", [P, NSP])
    e.hT_sl = ei("hT_sl", [P, NSP])
    e.h_sl = ei("h_sl", [NSP, DH])
    e.attT = ei("attT", [DE, EPC])
    e.idx16 = ei("idx16", [NT, P, EPT // 16], i16)
    e.colloc = ei("colloc", [NT, P, T_max])
    e.bmat = ei("bmat", [NT, P, G])
    srcs = dict(
        WxA=ei("WxA", [128, 129]), WhA=ei("WhA", [128, 129]),
        bA=ei("bA", [1, 129]), WqA=ei("WqA", [64, 129]),
        Wuh=ei("Wuh", [128, 128]), Wuc=ei("Wuc", [128, 128]),
        Wux=ei("Wux", [128, 128]), bu=ei("bu", [1, 128]),
        wron=ei("wron", [128, 1]), wrox=ei("wrox", [128, 1]),
        Wsc=ei("Wsc", [128, 1]), bsc=ei("bsc", [1, G]),
        iota=ei("iota", [P, P]), ident=ei("ident", [P, P]),
    )
    e.nf_out = nc.dram_tensor("nf_out", [NSP, DH], f32, kind="ExternalOutput")
    e.conf_out = nc.dram_tensor("conf_out", [1, G], f32, kind="ExternalOutput")

    with tile.TileContext(nc) as tc:
        with (
            tc.tile_pool(name="const", bufs=1) as cpool,
            tc.tile_pool(name="aslab", bufs=2) as apool,
            tc.tile_pool(name="asb", bufs=4) as asb,
            tc.tile_pool(name="edge", bufs=2) as epool,
            tc.tile_pool(name="gat", bufs=2) as gpool,
            tc.tile_pool(name="node", bufs=3) as npool,
            tc.tile_pool(name="psM", bufs=2, space="PSUM") as psM,
            tc.tile_pool(name="psQ", bufs=1, space="PSUM") as psQ,
            tc.tile_pool(name="psC", bufs=1, space="PSUM") as psC,
            tc.tile_pool(name="dram", bufs=1, space="DRAM") as dpool,
        ):
            e.cpool, e.apool, e.asb = cpool, apool, asb
            e.epool, e.gpool, e.npool = epool, gpool, npool
            e.psM, e.psQ, e.psC, e.dpool = psM, psQ, psC, dpool

            nc.gpsimd.load_library(mlp)

            for nm, src in srcs.items():
                t = cpool.tile(list(src.shape), f32, tag=nm)
                nc.sync.dma_start(t[:], src[:])
                setattr(e, nm + "_t", t)
            e.ones_t = cpool.tile([1, 128], f32, tag="ones")
            nc.vector.memset(e.ones_t[:], 1.0)
            e.gf_a = cpool.tile([G, 129], f32, tag="gfa")
            e.gf_b = cpool.tile([G, 129], f32, tag="gfb")
            e.table = dpool.tile([NPAD, ELEM], f32)

            loop_ctx = (tc.For_i(0, repeat, 1) if repeat > 1
                        else contextlib.nullcontext())
            with loop_ctx:
                _emit_body(e)
            _emit_tail(e)

    nc.compile()
    return nc


_CACHE = {}


def kernel(**inputs):
    num_graphs = int(np.asarray(inputs["num_graphs"]))
    assert num_graphs == G
    in_maps, T_max = _host_prep(
        inputs["x"], inputs["hidden_node_feat"], inputs["edge_attr"],
        inputs["W_emb"], inputs["b_emb"], inputs["w_agg"], inputs["W_upd"],
        inputs["b_upd"], inputs["w_ro"], inputs["W_score"], inputs["b_score"],
        inputs["edge_index"], inputs["batch"])
    if T_max not in _CACHE:
        _CACHE[T_max] = _build_nc(T_max)
    nc = _CACHE[T_max]
    res = run_bass_kernel_spmd(nc, in_maps, core_ids=list(range(NCORES)))
    node_feat = np.concatenate(
        [res.results[k]["nf_out"][:NS] for k in range(NCORES)], axis=0)
    confidence = res.results[0]["conf_out"].reshape(G, 1)
    return node_feat, confidence


# revision 7
# speedup vs baseline: 13.0862x; 1.1437x over previous
import sys

sys.path.insert(0, "/opt/trn_rl_repo")

import contextlib
import numpy as np
import concourse.bass as bass
import concourse.bacc as bacc
import concourse.mybir as mybir
import concourse.tile as tile
from concourse.bass_utils import run_bass_kernel_spmd
from concourse.library_config import mlp

P = 128
N = 20000
E = 320000
G = 64
DX = 128
DH = 128
DE = 64
NCORES = 8
NS = N // NCORES            # nodes per core (2500)
NT = (NS + P - 1) // P      # node tiles per core (20)
NSP = NT * P                # padded nodes per core (2560)
NPAD = NCORES * NSP         # padded table rows (20480)
ASLAB = 2048                # stage-A slab width
ELEM = 192                  # gather row width in f32 (768B, %256)

f32 = mybir.dt.float32
i16 = mybir.dt.int16


def _host_prep(x, hidden, edge_attr, W_emb, b_emb, w_agg, W_upd, b_upd,
               w_ro, W_score, b_score, edge_index, batch):
    rol = np.asarray(edge_index[0], dtype=np.int64)
    col = np.asarray(edge_index[1], dtype=np.int64)
    x = np.asarray(x, dtype=np.float32)
    hidden = np.asarray(hidden, dtype=np.float32)
    edge_attr = np.asarray(edge_attr, dtype=np.float32)
    batch = np.asarray(batch, dtype=np.int64)

    perm = np.argsort(col, kind="stable")
    rol_s = rol[perm]
    col_s = col[perm]
    ea_s = edge_attr[perm]

    # edges sorted by col -> contiguous slice per (core, node-tile)
    bounds = np.searchsorted(col_s, np.arange(0, NCORES * NS + 1))
    grp_lo = np.empty((NCORES, NT), dtype=np.int64)
    grp_hi = np.empty((NCORES, NT), dtype=np.int64)
    for k in range(NCORES):
        for j in range(NT):
            lo_node = k * NS + j * P
            hi_node = min(k * NS + (j + 1) * P, (k + 1) * NS)
            grp_lo[k, j] = bounds[lo_node]
            grp_hi[k, j] = bounds[hi_node]
    cnt = grp_hi - grp_lo
    T_max = max(int(np.max((cnt + P - 1) // P)), 1)
    EPT = T_max * P           # padded edges per node tile
    EPC = NT * EPT            # padded edges per core

    # gather position i -> (partition i%128, block i//128);
    # idx tensor layout: idx i at [i % 16, i // 16], tiled to 128 partitions
    attT = np.zeros((NCORES, DE, EPC), dtype=np.float32)
    idx16 = np.zeros((NCORES, NT, P, EPT // 16), dtype=np.int16)
    colloc = np.full((NCORES, NT, P, T_max), -1.0, dtype=np.float32)
    for k in range(NCORES):
        for j in range(NT):
            lo, hi = grp_lo[k, j], grp_hi[k, j]
            n = int(hi - lo)
            base = j * EPT
            if n:
                attT[k, :, base:base + n] = ea_s[lo:hi].T
            r = np.zeros(EPT, dtype=np.int64)
            r[:n] = rol_s[lo:hi]
            # map global node id -> padded table row
            rr = (r // NS) * NSP + (r % NS)
            blk = rr.reshape(EPT // 16, 16).T.astype(np.int16)
            idx16[k, j] = np.tile(blk, (8, 1))
            cl = np.full(EPT, -1.0, dtype=np.float32)
            cl[:n] = (col_s[lo:hi] - (k * NS + j * P)).astype(np.float32)
            colloc[k, j] = cl.reshape(T_max, P).T                  # [P, T_max]

    xT_full = np.zeros((P, NPAD), dtype=np.float32)
    hT_full = np.zeros((P, NPAD), dtype=np.float32)
    for k in range(NCORES):
        xT_full[:, k * NSP:k * NSP + NS] = x[k * NS:(k + 1) * NS].T
        hT_full[:, k * NSP:k * NSP + NS] = hidden[k * NS:(k + 1) * NS].T

    bmat = np.zeros((NCORES, NT, P, G), dtype=np.float32)
    for k in range(NCORES):
        b = batch[k * NS:(k + 1) * NS]
        oh = np.zeros((NSP, G), dtype=np.float32)
        oh[np.arange(NS), b] = 1.0
        bmat[k] = oh.reshape(NT, P, G)

    W_emb = np.asarray(W_emb, dtype=np.float32)
    w_agg = np.asarray(w_agg, dtype=np.float32)
    W_upd = np.asarray(W_upd, dtype=np.float32)
    w_ro = np.asarray(w_ro, dtype=np.float32)

    weights = dict(
        WxA=np.ascontiguousarray(
            np.concatenate([W_emb[0:128], w_agg[0:128]], axis=1)),      # [128,129]
        WhA=np.ascontiguousarray(
            np.concatenate([W_emb[128:256], w_agg[128:256]], axis=1)),  # [128,129]
        bA=np.concatenate([np.asarray(b_emb, np.float32),
                           np.zeros(1, np.float32)])[None, :],          # [1,129]
        WqA=np.ascontiguousarray(
            np.concatenate([W_emb[256:320], w_agg[512:576]], axis=1)),  # [64,129]
        Wuh=np.ascontiguousarray(W_upd[0:128]),
        Wuc=np.ascontiguousarray(W_upd[128:256]),
        Wux=np.ascontiguousarray(W_upd[256:384]),
        bu=np.asarray(b_upd, np.float32)[None, :],                      # [1,128]
        wron=np.ascontiguousarray(w_ro[0:128]),
        wrox=np.ascontiguousarray(w_ro[128:256]),                       # [128,1]
        Wsc=np.asarray(W_score, np.float32),                            # [128,1]
        bsc=np.full((1, G), float(np.asarray(b_score).reshape(-1)[0]),
                    np.float32),                                        # [1,G]
        iota=np.tile(np.arange(P, dtype=np.float32), (P, 1)),           # [128,128]
        ident=np.eye(P, dtype=np.float32),                              # [128,128]
    )

    in_maps = []
    for k in range(NCORES):
        m = dict(weights)
        m["xT_full"] = xT_full
        m["hT_full"] = hT_full
        m["xT_sl"] = np.ascontiguousarray(xT_full[:, k * NSP:(k + 1) * NSP])
        m["hT_sl"] = np.ascontiguousarray(hT_full[:, k * NSP:(k + 1) * NSP])
        m["h_sl"] = np.ascontiguousarray(
            np.vstack([hidden[k * NS:(k + 1) * NS],
                       np.zeros((NSP - NS, DH), np.float32)]))
        m["attT"] = attT[k]
        m["idx16"] = idx16[k]
        m["colloc"] = colloc[k]
        m["bmat"] = bmat[k]
        in_maps.append(m)
    return in_maps, T_max


class _Env:
    pass


def _emit_body(e):
    """Stage A (node table) + stage B (edge pipeline + node stage)."""
    nc, T_max, EPT, NB = e.nc, e.T_max, e.EPT, e.NB
    e.cn_tiles = []
    AF = mybir.ActivationFunctionType
    OP = mybir.AluOpType

    # ---------------- stage A: node table [P | a_src] ----------------
    for s in range(NPAD // ASLAB):
        xs = e.apool.tile([P, ASLAB], f32, tag="xs")
        hs = e.apool.tile([P, ASLAB], f32, tag="hs")
        nc.sync.dma_start(xs[:], e.xT_full[:, s * ASLAB:(s + 1) * ASLAB])
        nc.sync.dma_start(hs[:], e.hT_full[:, s * ASLAB:(s + 1) * ASLAB])
        for t in range(ASLAB // P):
            ps = e.psM.tile([P, 129], f32, space="PSUM", tag="misc")
            nc.tensor.matmul(ps[:], xs[:, t * P:(t + 1) * P], e.WxA_t[:],
                             start=True, stop=False)
            nc.tensor.matmul(ps[:], hs[:, t * P:(t + 1) * P], e.WhA_t[:],
                             start=False, stop=False)
            nc.tensor.matmul(ps[:], e.ones_t[:1, :], e.bA_t[:],
                             start=False, stop=True)
            sa = e.asb.tile([P, 129], f32, tag="sa")
            nc.scalar.activation(sa[:], ps[:], AF.Copy)
            r0 = s * ASLAB + t * P
            nc.sync.dma_start(e.table[r0:r0 + P, 0:129], sa[:])

    # ---------------- stage B: edge pipeline ----------------
    for j in range(NT):
        idx_t = e.epool.tile([P, EPT // 16], i16, tag="idx")
        nc.sync.dma_start(idx_t[:], e.idx16[j])
        cj_t = e.epool.tile([P, T_max], f32, tag="cj")
        nc.sync.dma_start(cj_t[:], e.colloc[j])
        at_t = e.epool.tile([DE, EPT], f32, tag="at")
        nc.sync.dma_start(at_t[:], e.attT[:, j * EPT:(j + 1) * EPT])
        g_t = e.gpool.tile([P, T_max, ELEM], f32, tag="g")
        nc.gpsimd.dma_gather(g_t[:], e.table[:, :], idx_t[:], EPT, EPT, ELEM,
                             single_packet=False)

        # q matmuls: 4 tiles of [128,128] per PSUM bank; ae into c_ps spare
        q_ps = e.psQ.tile([P, NB * 512], f32, space="PSUM", tag="q")
        c_ps = e.psC.tile([P, 512], f32, space="PSUM", tag="c")
        for t in range(T_max):
            off = (t // 4) * 512 + (t % 4) * 128
            nc.tensor.matmul(q_ps[:, off:off + 128],
                             at_t[:, t * P:(t + 1) * P], e.WqA_t[:, 0:128],
                             start=True, stop=True)
            nc.tensor.matmul(c_ps[:, 384 + t:385 + t],
                             at_t[:, t * P:(t + 1) * P], e.WqA_t[:, 128:129],
                             start=True, stop=True, skip_group_check=True)

        # V = P[rol] + q   (wide adds over 4-tile bank groups)
        v_t = e.gpool.tile([P, T_max * P], f32, tag="v")
        n4 = T_max // 4
        r4 = T_max - n4 * 4
        if n4:
            nc.vector.tensor_tensor(
                out=v_t[:, 0:n4 * 512].rearrange(
                    "p (a b c) -> p a b c", a=n4, b=4),
                in0=q_ps[:, 0:n4 * 512].rearrange(
                    "p (a b c) -> p a b c", a=n4, b=4),
                in1=g_t[:, 0:n4 * 4, 0:128].rearrange(
                    "p (a b) c -> p a b c", a=n4),
                op=OP.add)
        if r4:
            nc.vector.tensor_tensor(
                out=v_t[:, n4 * 512:].rearrange("p (a c) -> p a c", a=r4),
                in0=q_ps[:, n4 * 512:n4 * 512 + r4 * 128].rearrange(
                    "p (a c) -> p a c", a=r4),
                in1=g_t[:, n4 * 4:, 0:128],
                op=OP.add)

        # att = a_src + ae ; alpha = exp(att)
        att_t = e.epool.tile([P, T_max], f32, tag="att")
        nc.vector.tensor_tensor(
            out=att_t[:], in0=g_t[:, :, 128],
            in1=c_ps[:, 384:384 + T_max], op=OP.add)
        al_t = e.epool.tile([P, T_max], f32, tag="al")
        nc.scalar.activation(al_t[:], att_t[:], AF.Exp)

        # S = onehot(col_local), one wide op
        s_t = e.gpool.tile([P, T_max * P], f32, tag="s")
        nc.vector.tensor_tensor(
            out=s_t[:].rearrange("p (a b) -> p a b", a=T_max),
            in0=e.iota_t[:].unsqueeze(1).broadcast_to([P, T_max, P]),
            in1=cj_t[:].to_broadcast([P, T_max, P]),
            op=OP.is_equal)

        # Vt = relu(alpha * V); scatter-matmuls into [C | z]
        vt_t = e.gpool.tile([P, T_max * P], f32, tag="vt")
        for t in range(T_max):
            nc.scalar.activation(vt_t[:, t * P:(t + 1) * P],
                                 v_t[:, t * P:(t + 1) * P], AF.Relu,
                                 scale=al_t[:, t:t + 1])
            # start=True clears the whole bank's has_written bits, so only
            # the first matmul touching this bank may set it.
            nc.tensor.matmul(c_ps[:, 0:128], s_t[:, t * P:(t + 1) * P],
                             vt_t[:, t * P:(t + 1) * P],
                             start=(t == 0), stop=False,
                             skip_group_check=True)
            nc.tensor.matmul(c_ps[:, 128:129], s_t[:, t * P:(t + 1) * P],
                             al_t[:, t:t + 1],
                             start=False, stop=(t == T_max - 1),
                             skip_group_check=True)

        # divide and park C for pass 2
        zr = e.npool.tile([P, 1], f32, tag="zr")
        nc.vector.tensor_scalar_add(zr[:], c_ps[:, 128:129], 1e-16)
        nc.vector.reciprocal(zr[:], zr[:])
        cn = e.cnpool.tile([P, 128], f32, tag=f"cn{j}")
        nc.vector.tensor_scalar_mul(cn[:], c_ps[:, 0:128], zr[:])
        e.cn_tiles.append(cn)

    # ---------------- pass 2: node stage ----------------
    for j in range(NT):
        cn = e.cn_tiles[j]
        ct_ps = e.psM.tile([P, 128], f32, space="PSUM", tag="misc")
        nc.tensor.transpose(ct_ps[:], cn[:], e.ident_t[:])
        ct = e.npool.tile([P, 128], f32, tag="ct")
        nc.scalar.activation(ct[:], ct_ps[:], AF.Copy)

        ht_j = e.npool.tile([P, 128], f32, tag="htj")
        nc.sync.dma_start(ht_j[:], e.hT_sl[:, j * P:(j + 1) * P])
        xt_j = e.npool.tile([P, 128], f32, tag="xtj")
        nc.sync.dma_start(xt_j[:], e.xT_sl[:, j * P:(j + 1) * P])
        h_j = e.npool.tile([P, 128], f32, tag="hj")
        nc.sync.dma_start(h_j[:], e.h_sl[j * P:(j + 1) * P, :])

        g_ps = e.psM.tile([P, 128], f32, space="PSUM", tag="misc")
        nc.tensor.matmul(g_ps[:], ht_j[:], e.Wuh_t[:], start=True, stop=False)
        nc.tensor.matmul(g_ps[:], ct[:], e.Wuc_t[:], start=False, stop=False)
        nc.tensor.matmul(g_ps[:], xt_j[:], e.Wux_t[:], start=False, stop=False)
        nc.tensor.matmul(g_ps[:], e.ones_t[:1, :], e.bu_t[:],
                         start=False, stop=True)
        gate = e.npool.tile([P, 128], f32, tag="gate")
        nc.scalar.activation(gate[:], g_ps[:], AF.Sigmoid)

        d_t = e.npool.tile([P, 128], f32, tag="d")
        nc.vector.tensor_tensor(out=d_t[:], in0=cn[:], in1=h_j[:],
                                op=OP.subtract)
        nf1 = e.npool.tile([P, 128], f32, tag="nf1")
        nc.vector.tensor_tensor(out=nf1[:], in0=gate[:], in1=d_t[:],
                                op=OP.mult)
        nf2 = e.npool.tile([P, 128], f32, tag="nf2")
        nc.vector.tensor_tensor(out=nf2[:], in0=nf1[:], in1=h_j[:],
                                op=OP.add)
        nc.sync.dma_start(e.nf_out[j * P:(j + 1) * P, :], nf2[:])

        nft_ps = e.psM.tile([P, 128], f32, space="PSUM", tag="misc")
        nc.tensor.transpose(nft_ps[:], nf2[:], e.ident_t[:])
        nft = e.npool.tile([P, 128], f32, tag="nft")
        nc.scalar.activation(nft[:], nft_ps[:], AF.Copy)

        ro_ps = e.psM.tile([P, 1], f32, space="PSUM", tag="misc")
        nc.tensor.matmul(ro_ps[:], nft[:], e.wron_t[:], start=True, stop=False)
        nc.tensor.matmul(ro_ps[:], xt_j[:], e.wrox_t[:], start=False, stop=True)
        ero = e.npool.tile([P, 1], f32, tag="ero")
        nc.scalar.activation(ero[:], ro_ps[:], AF.Exp)

        r_t = e.npool.tile([P, 129], f32, tag="rt")
        nc.vector.tensor_scalar_mul(r_t[:, 0:128], nf2[:], ero[:])
        nc.vector.tensor_copy(r_t[:, 128:129], ero[:])

        bj = e.npool.tile([P, G], f32, tag="bj")
        nc.sync.dma_start(bj[:], e.bmat[j])
        gfj_ps = e.psM.tile([G, 129], f32, space="PSUM", tag="misc")
        nc.tensor.matmul(gfj_ps[:], bj[:], r_t[:], start=True, stop=True)
        if j == 0:
            nc.vector.tensor_copy(e.gf_b[:], gfj_ps[:])
        elif j % 2 == 1:
            nc.vector.tensor_tensor(out=e.gf_a[:], in0=e.gf_b[:],
                                    in1=gfj_ps[:], op=OP.add)
        else:
            nc.vector.tensor_tensor(out=e.gf_b[:], in0=e.gf_a[:],
                                    in1=gfj_ps[:], op=OP.add)


def _emit_tail(e):
    nc = e.nc
    AF = mybir.ActivationFunctionType
    OP = mybir.AluOpType
    gf_fin = e.gf_a if NT % 2 == 0 else e.gf_b

    ar_in = e.dpool.tile([G, 129], f32)
    ar_out = e.dpool.tile([G, 129], f32)
    nc.sync.dma_start(ar_in[:], gf_fin[:])
    nc.gpsimd.collective_compute(
        "AllReduce", OP.add, replica_groups=[list(range(NCORES))],
        ins=[ar_in.opt()], outs=[ar_out.opt()])
    gfr = e.npool.tile([G, 129], f32, tag="gfr")
    nc.sync.dma_start(gfr[:], ar_out[:])

    zg = e.npool.tile([G, 1], f32, tag="zg")
    nc.vector.tensor_scalar_add(zg[:], gfr[:, 128:129], 1e-16)
    nc.vector.reciprocal(zg[:], zg[:])
    gf = e.npool.tile([G, 128], f32, tag="gf")
    nc.vector.tensor_scalar_mul(gf[:], gfr[:, 0:128], zg[:])

    gft_ps = e.psM.tile([P, G], f32, space="PSUM", tag="misc")
    nc.tensor.transpose(gft_ps[:], gf[:], e.ident_t[0:G, 0:G])
    gft = e.npool.tile([P, G], f32, tag="gft")
    nc.scalar.activation(gft[:], gft_ps[:], AF.Copy)

    cf_ps = e.psM.tile([1, G], f32, space="PSUM", tag="misc")
    nc.tensor.matmul(cf_ps[:], e.Wsc_t[:], gft[:], start=True, stop=False)
    nc.tensor.matmul(cf_ps[:], e.ones_t[:1, :1], e.bsc_t[:],
                     start=False, stop=True)
    conf = e.npool.tile([1, G], f32, tag="conf")
    nc.scalar.activation(conf[:], cf_ps[:], AF.Sigmoid)
    nc.sync.dma_start(e.conf_out[:], conf[:])


def _build_nc(T_max, repeat=1):
    EPT = T_max * P
    NB = (T_max + 3) // 4       # q PSUM banks (4 tiles of 128 per bank)
    EPC = NT * EPT
    nc = bacc.Bacc("TRN2", target_bir_lowering=False, debug=False,
                   num_devices=NCORES)
    e = _Env()
    e.nc, e.T_max, e.EPT, e.NB = nc, T_max, EPT, NB

    ei = lambda nm, sh, dt=f32: nc.dram_tensor(nm, sh, dt, kind="ExternalInput")
    e.xT_full = ei("xT_full", [P, NPAD])
    e.hT_full = ei("hT_full", [P, NPAD])
    e.xT_sl = ei("xT_sl", [P, NSP])
    e.hT_sl = ei("hT_sl", [P, NSP])
    e.h_sl = ei("h_sl", [NSP, DH])
    e.attT = ei("attT", [DE, EPC])
    e.idx16 = ei("idx16", [NT, P, EPT // 16], i16)
    e.colloc = ei("colloc", [NT, P, T_max])
    e.bmat = ei("bmat", [NT, P, G])
    srcs = dict(
        WxA=ei("WxA", [128, 129]), WhA=ei("WhA", [128, 129]),
        bA=ei("bA", [1, 129]), WqA=ei("WqA", [64, 129]),
        Wuh=ei("Wuh", [128, 128]), Wuc=ei("Wuc", [128, 128]),
        Wux=ei("Wux", [128, 128]), bu=ei("bu", [1, 128]),
        wron=ei("wron", [128, 1]), wrox=ei("wrox", [128, 1]),
        Wsc=ei("Wsc", [128, 1]), bsc=ei("bsc", [1, G]),
        iota=ei("iota", [P, P]), ident=ei("ident", [P, P]),
    )
    e.nf_out = nc.dram_tensor("nf_out", [NSP, DH], f32, kind="ExternalOutput")
    e.conf_out = nc.dram_tensor("conf_out", [1, G], f32, kind="ExternalOutput")

    with tile.TileContext(nc) as tc:
        with (
            tc.tile_pool(name="const", bufs=1) as cpool,
            tc.tile_pool(name="aslab", bufs=2) as apool,
            tc.tile_pool(name="asb", bufs=4) as asb,
            tc.tile_pool(name="edge", bufs=2) as epool,
            tc.tile_pool(name="gat", bufs=2) as gpool,
            tc.tile_pool(name="node", bufs=3) as npool,
            tc.tile_pool(name="cn", bufs=1) as cnpool,
            tc.tile_pool(name="psM", bufs=2, space="PSUM") as psM,
            tc.tile_pool(name="psQ", bufs=1, space="PSUM") as psQ,
            tc.tile_pool(name="psC", bufs=1, space="PSUM") as psC,
            tc.tile_pool(name="dram", bufs=1, space="DRAM") as dpool,
        ):
            e.cpool, e.apool, e.asb = cpool, apool, asb
            e.epool, e.gpool, e.npool = epool, gpool, npool
            e.cnpool = cnpool
            e.psM, e.psQ, e.psC, e.dpool = psM, psQ, psC, dpool

            nc.gpsimd.load_library(mlp)

            for nm, src in srcs.items():
                t = cpool.tile(list(src.shape), f32, tag=nm)
                nc.sync.dma_start(t[:], src[:])
                setattr(e, nm + "_t", t)
            e.ones_t = cpool.tile([1, 128], f32, tag="ones")
            nc.vector.memset(e.ones_t[:], 1.0)
            e.gf_a = cpool.tile([G, 129], f32, tag="gfa")
            e.gf_b = cpool.tile([G, 129], f32, tag="gfb")
            e.cn_tiles = []
            e.table = dpool.tile([NPAD, ELEM], f32)

            loop_ctx = (tc.For_i(0, repeat, 1) if repeat > 1
                        else contextlib.nullcontext())
            with loop_ctx:
                _emit_body(e)
            _emit_tail(e)

    nc.compile()
    return nc


_CACHE = {}


def kernel(**inputs):
    num_graphs = int(np.asarray(inputs["num_graphs"]))
    assert num_graphs == G
    in_maps, T_max = _host_prep(
        inputs["x"], inputs["hidden_node_feat"], inputs["edge_attr"],
        inputs["W_emb"], inputs["b_emb"], inputs["w_agg"], inputs["W_upd"],
        inputs["b_upd"], inputs["w_ro"], inputs["W_score"], inputs["b_score"],
        inputs["edge_index"], inputs["batch"])
    if T_max not in _CACHE:
        _CACHE[T_max] = _build_nc(T_max)
    nc = _CACHE[T_max]
    res = run_bass_kernel_spmd(nc, in_maps, core_ids=list(range(NCORES)))
    node_feat = np.concatenate(
        [res.results[k]["nf_out"][:NS] for k in range(NCORES)], axis=0)
    confidence = res.results[0]["conf_out"].reshape(G, 1)
    return node_feat, confidence
